# revision 12
# baseline (speedup 1.0000x reference)
"""Trainium2 Bass kernel for nn_NodewiseMassBias.

Pipeline per core (2 batches of the 16, data-parallel over batch):
  for each query tile of 128 tokens:
    PE   : Lorentz-dot slab [128, 2048] (mask folded in as 5th contraction row)
    ACT  : slab PSUM -> SBUF
    DVE  : max8 (top-8 values per row) + max_index (their column indices)
    SWDGE: indirect gather of the 8 friends' masked 4-momenta from DRAM
    DVE  : prefix sums over sorted friends -> invariant masses for k=2,4,8
  then one Sqrt pass and a per-tile MLP (bias via extra rank-1 matmuls),
  GELU on ACT, output DMA per tile.
"""

import math

import numpy as np

NCORES = 8
B, T, D, H = 16, 2048, 512, 64
NB = B // NCORES  # batches per core
P = 128
NT = T // P
NK = 3
KV = (2, 4, 8)

_PROGRAMS = {}


def _build_program(masked, debug=False):
    import concourse.bass as bass
    import concourse.mybir as mybir
    from concourse.masks import make_identity
    from concourse.tile import TileContext

    f32 = mybir.dt.float32
    u8 = mybir.dt.uint8
    u32 = mybir.dt.uint32
    AF = mybir.ActivationFunctionType
    ALU = mybir.AluOpType
    AX = mybir.AxisListType

    LN_HALF = float(math.log(0.5))
    HALF_PI = float(math.pi / 2.0)
    PI_F = float(np.float32(math.pi))
    NEG_BIG = -1.0e30
    SQ2H = float(math.sqrt(2.0) / 2.0)

    nc = bass.Bass()
    tokens = nc.dram_tensor("tokens", [NB, T, 4], f32, kind="ExternalInput")
    maskin = nc.dram_tensor("maskin", [NB, T], u8, kind="ExternalInput")
    w1 = nc.dram_tensor("w1", [NK, H], f32, kind="ExternalInput")
    b1 = nc.dram_tensor("b1", [H], f32, kind="ExternalInput")
    w2 = nc.dram_tensor("w2", [H, D], f32, kind="ExternalInput")
    b2 = nc.dram_tensor("b2", [D], f32, kind="ExternalInput")
    outd = nc.dram_tensor("out", [NB, T, D], f32, kind="ExternalOutput")
    p0d = [nc.dram_tensor(f"p0_{bb}", [T, 4], f32) for bb in range(NB)]
    if debug:
        dbg_maxv = nc.dram_tensor("dbg_maxv", [NB, NT, P, 8], f32, kind="ExternalOutput")
        dbg_idx = nc.dram_tensor("dbg_idx", [NB, NT, P, 8], u32, kind="ExternalOutput")
        dbg_pf = nc.dram_tensor("dbg_pf", [NB, NT, P, 32], f32, kind="ExternalOutput")
        dbg_mass = nc.dram_tensor("dbg_mass", [NB, T, NK], f32, kind="ExternalOutput")

    with TileContext(nc) as tc:
        with (
            tc.tile_pool(name="consts", bufs=1) as consts,
            tc.tile_pool(name="bwork", bufs=2) as bwork,
            tc.tile_pool(name="twork", bufs=2) as twork,
            tc.tile_pool(name="slabpool", bufs=1, space="PSUM") as slabpool,
            tc.tile_pool(name="mlppool", bufs=1, space="PSUM") as mlppool,
        ):
            ident = consts.tile([P, P], f32)
            make_identity(nc, ident)
            # ones row built on ACT (slab bias matmul then only waits ACT).
            ones1 = consts.tile([1, P], f32)
            nc.scalar.activation(ones1, ident[0:1, :], AF.Copy, scale=0.0, bias=1.0)

            _cbias_cache = {}

            def cbias(val, nparts=P):
                if val not in _cbias_cache:
                    t = consts.tile([P, 1], f32, tag=f"cbias_{len(_cbias_cache)}")
                    nc.vector.memset(t, float(val))
                    _cbias_cache[val] = t
                return _cbias_cache[val][:nparts, :]
            w1s = consts.tile([NK, H], f32)
            nc.sync.dma_start(w1s, w1[:])
            b1s = consts.tile([1, H], f32)
            nc.sync.dma_start(b1s, b1.rearrange("(o h) -> o h", o=1))
            w2s = consts.tile([H, D], f32)
            nc.sync.dma_start(w2s, w2[:])
            b2s = consts.tile([1, D], f32)
            nc.sync.dma_start(b2s, b2.rearrange("(o d) -> o d", o=1))
            # Warm-up matmuls: let PE observe each producer semaphore here,
            # one wait per instruction, so real matmuls carry <=1 wait
            # (PE LDWEIGHTS encodes at most one sync wait).
            warm = mlppool.tile([P, 128], f32, tag="tp")
            nc.tensor.transpose(warm[:P, :P], ident, ident)
            nc.tensor.matmul(warm[:H, :H], lhsT=w1s, rhs=w1s, start=True, stop=True)
            nc.tensor.matmul(warm[:H, :H], lhsT=b1s, rhs=b1s, start=True, stop=True)
            nc.tensor.matmul(
                warm[:, :128], lhsT=w2s[:, 0:P], rhs=w2s[:, 0:P], start=True, stop=True
            )
            nc.tensor.matmul(warm[:P, :P], lhsT=b2s[:, 0:P], rhs=b2s[:, 0:P], start=True, stop=True)

            for bi in range(NB):
                # ---- per-batch prep ----
                tok = bwork.tile([P, NT, 4], f32, tag="tok")
                nc.sync.dma_start(tok, tokens[bi].rearrange("(n p) c -> p n c", p=P))
                if masked:
                    mku = bwork.tile([P, NT], u8, tag="mku")
                    nc.sync.dma_start(mku, maskin[bi].rearrange("(n p) -> p n", p=P))
                    maskf = bwork.tile([P, NT], f32, tag="maskf")
                    nc.vector.tensor_copy(maskf, mku)
                    mrow_u = bwork.tile([1, T], u8, tag="mrowu")
                    nc.sync.dma_start(mrow_u, maskin[bi].rearrange("(o t) -> o t", o=1))
                    mrow_f = bwork.tile([1, T], f32, tag="mrowf")
                    nc.vector.tensor_copy(mrow_f, mrow_u)

                Pn = bwork.tile([P, NT, 4], f32, tag="Pn")
                P0 = bwork.tile([P, NT, 4], f32, tag="P0")
                etac = bwork.tile([P, NT], f32, tag="etac")
                expp = bwork.tile([P, NT], f32, tag="expp")
                expm = bwork.tile([P, NT], f32, tag="expm")
                cosp = bwork.tile([P, NT], f32, tag="cosp")
                sinp = bwork.tile([P, NT], f32, tag="sinp")
                shh = bwork.tile([P, NT], f32, tag="shh")
                phic = bwork.tile([P, NT], f32, tag="phic")
                aphi = bwork.tile([P, NT], f32, tag="aphi")

                nc.vector.tensor_scalar(
                    etac, tok[:, :, 2], -20.0, 20.0, op0=ALU.max, op1=ALU.min
                )
                nc.vector.tensor_scalar(
                    phic, tok[:, :, 3], -PI_F, PI_F, op0=ALU.max, op1=ALU.min
                )
                nc.scalar.activation(expp, etac, AF.Exp, bias=cbias(LN_HALF))
                nc.scalar.activation(expm, etac, AF.Exp, scale=-1.0, bias=cbias(LN_HALF))
                nc.scalar.activation(aphi, phic, AF.Abs)
                # cos(x) = sin(pi/2 - |x|), argument stays inside [-pi/2, pi/2]
                nc.scalar.activation(
                    cosp, aphi, AF.Sin, scale=-1.0, bias=cbias(HALF_PI)
                )
                nc.scalar.activation(sinp, phic, AF.Sin)
                nc.vector.tensor_copy(Pn[:, :, 0], tok[:, :, 0])
                nc.vector.tensor_mul(Pn[:, :, 1], tok[:, :, 1], cosp)
                nc.vector.tensor_mul(Pn[:, :, 2], tok[:, :, 1], sinp)
                nc.vector.tensor_sub(shh, expp, expm)
                nc.vector.tensor_mul(Pn[:, :, 3], tok[:, :, 1], shh)
                if masked:
                    for c in range(4):
                        nc.vector.tensor_mul(P0[:, :, c], Pn[:, :, c], maskf)
                    nc.sync.dma_start(p0d[bi].rearrange("(n p) c -> p n c", p=P), P0)
                else:
                    nc.sync.dma_start(p0d[bi].rearrange("(n p) c -> p n c", p=P), Pn)

                # Transposed layouts for the dot matmuls:
                # PTe = [E,px,py,pz], PTme = [E,-px,-py,-pz], both [4, T].
                Pnm = bwork.tile([P, NT, 4], f32, tag="Pnm")
                nc.vector.tensor_copy(Pnm[:, :, 0:1], Pn[:, :, 0:1])
                nc.vector.tensor_scalar_mul(Pnm[:, :, 1:4], Pn[:, :, 1:4], -1.0)
                PTe = bwork.tile([4, T], f32, tag="PTe")
                PTme = bwork.tile([4, T], f32, tag="PTme")
                for j in range(NT):
                    tpe = mlppool.tile([4, P], f32, tag="tp")
                    nc.tensor.transpose(tpe, Pn[:, j, :], ident)
                    nc.scalar.copy(PTe[:, j * P : (j + 1) * P], tpe)
                    tpm = mlppool.tile([4, P], f32, tag="tp")
                    nc.tensor.transpose(tpm, Pnm[:, j, :], ident)
                    nc.scalar.copy(PTme[:, j * P : (j + 1) * P], tpm)
                if masked:
                    # Fold the column mask into PTme's E row: E'_s = E_s - 1e30*(1-m_s)
                    # makes dot'[t,s] = dot[t,s] - E_t*1e30*(1-m_s) << any valid dot
                    # (E_t > 0), so masked s never enter the top-8.
                    mneg = bwork.tile([1, T], f32, tag="mneg")
                    nc.scalar.activation(
                        mneg, mrow_f, AF.Identity, scale=1.0e30, bias=cbias(NEG_BIG, 1)
                    )
                    nc.vector.tensor_add(PTme[0:1, :], PTme[0:1, :], mneg)

                # ---- loop 1: dot slab, top-8, friend sums, masses ----
                m2all = bwork.tile([P, NT, NK], f32, tag="m2all")
                massall = bwork.tile([P, NT, NK], f32, tag="massall")
                for i in range(NT):
                    slab = slabpool.tile([P, T], f32, tag="slab")
                    for j in range(4):
                        nc.tensor.matmul(
                            slab[:, j * 512 : (j + 1) * 512],
                            lhsT=PTe[:, i * P : (i + 1) * P],
                            rhs=PTme[:, j * 512 : (j + 1) * 512],
                            start=True,
                            stop=True,
                        )
                    ssb = twork.tile([P, T], f32, tag="ssb")
                    nc.scalar.copy(ssb, slab)
                    mxv = twork.tile([P, 8], f32, tag="mxv")
                    nc.vector.max(out=mxv, in_=ssb)
                    idx = twork.tile([P, 8], u32, tag="idx")
                    nc.vector.max_index(out=idx, in_max=mxv, in_values=ssb)
                    pf = twork.tile([P, 8, 4], f32, tag="pf")
                    # one indirect DMA per friend column: the [P,8,4]-out /
                    # [P,8]-offset form miscorresponds on hardware
                    for j in range(8):
                        nc.gpsimd.indirect_dma_start(
                            out=pf[:, j, :],
                            out_offset=None,
                            in_=p0d[bi][:],
                            in_offset=bass.IndirectOffsetOnAxis(
                                ap=idx[:, j : j + 1], axis=0
                            ),
                        )
                    ps = twork.tile([P, NK, 4], f32, tag="ps")
                    for kk, k in enumerate(KV):
                        nc.vector.reduce_sum(
                            ps[:, kk, :],
                            pf[:, 0:k, :].rearrange("p j c -> p c j"),
                            axis=AX.X,
                        )
                    q = twork.tile([P, NK, 4], f32, tag="q")
                    nc.vector.tensor_mul(q, ps, ps)
                    s3 = twork.tile([P, NK], f32, tag="s3")
                    nc.vector.reduce_sum(s3, q[:, :, 1:4], axis=AX.X)
                    nc.vector.tensor_sub(m2all[:, i, :], q[:, :, 0], s3)
                    if debug:
                        nc.sync.dma_start(dbg_maxv[bi, i], mxv)
                        nc.sync.dma_start(dbg_idx[bi, i], idx)
                        nc.sync.dma_start(dbg_pf[bi, i], pf.rearrange("p j c -> p (j c)"))
                    if masked:
                        nc.vector.tensor_scalar(
                            m2all[:, i, :],
                            m2all[:, i, :],
                            0.0,
                            maskf[:, i : i + 1],
                            op0=ALU.max,
                            op1=ALU.mult,
                        )
                    else:
                        nc.vector.tensor_scalar_max(
                            m2all[:, i, :], m2all[:, i, :], 0.0
                        )
                nc.scalar.activation(massall, m2all, AF.Sqrt, bias=cbias(1.0e-8))
                if debug:
                    nc.sync.dma_start(
                        dbg_mass[bi].rearrange("(n p) k -> p n k", p=P), massall
                    )

                # ---- loop 2: MLP ----
                for i in range(NT):
                    mtp = mlppool.tile([NK, P], f32, tag="tp")
                    nc.tensor.transpose(mtp, massall[:, i, :], ident)
                    mts = twork.tile([NK, P], f32, tag="mts")
                    nc.scalar.copy(mts, mtp)
                    h1p = mlppool.tile([P, H], f32, tag="h1p")
                    nc.tensor.matmul(h1p, lhsT=mts, rhs=w1s, start=True, stop=False)
                    nc.tensor.matmul(h1p, lhsT=ones1, rhs=b1s, start=False, stop=True)
                    uclip = twork.tile([P, H], f32, tag="uclip")
                    nc.vector.tensor_scalar(
                        uclip, h1p, 4.0, -4.0, op0=ALU.min, op1=ALU.max
                    )
                    erf_t = twork.tile([P, H], f32, tag="erf_t")
                    nc.scalar.activation(erf_t, uclip, AF.Erf)
                    phi_t = twork.tile([P, H], f32, tag="phi_t")
                    nc.vector.tensor_scalar(
                        phi_t, erf_t, SQ2H, SQ2H, op0=ALU.mult, op1=ALU.add
                    )
                    hsb = twork.tile([P, H], f32, tag="hsb")
                    nc.vector.tensor_mul(hsb, h1p, phi_t)
                    htp = mlppool.tile([H, P], f32, tag="htp")
                    nc.tensor.transpose(htp, hsb, ident)
                    hts = twork.tile([H, P], f32, tag="hts")
                    nc.scalar.copy(hts, htp)
                    op_ = mlppool.tile([P, D], f32, tag="opp")
                    nc.tensor.matmul(op_, lhsT=hts, rhs=w2s, start=True, stop=False)
                    nc.tensor.matmul(op_, lhsT=ones1, rhs=b2s, start=False, stop=True)
                    osb = twork.tile([P, D], f32, tag="osb")
                    nc.scalar.copy(osb, op_)
                    nc.sync.dma_start(
                        outd[bi].rearrange("(n p) d -> p n d", p=P)[:, i, :], osb
                    )
    return nc


def _legalize_sync_waits(nc):
    """This toolchain's walrus accepts at most ONE sync wait per instruction.

    Hoist surplus waits onto same-engine NoOps inserted immediately before
    the offending instruction (engine streams execute in order, so the NoOp
    waits gate the instruction just the same).
    """
    import concourse.mybir as mybir

    n_fixed = 0
    for fn in nc.m.functions:
        for bb in fn.blocks:
            insts = bb.instructions
            i = 0
            while i < len(insts):
                inst = insts[i]
                si = getattr(inst, "sync_info", None)
                eng = getattr(inst, "engine", None)
                if (
                    si is not None
                    and si.on_wait
                    and len(si.on_wait) > 1
                    and eng is not None
                    and eng != mybir.EngineType.Unassigned
                ):
                    extra = list(si.on_wait[:-1])
                    for j, w in enumerate(extra):
                        nop = mybir.InstNoOp(
                            name=f"{inst.name}-wsplit{j}", ins=[], outs=[]
                        )
                        nop.engine = eng
                        nop.sync_info = mybir.SyncInfo(on_wait=[w], on_update=[])
                        insts.insert(i, nop)
                        i += 1
                    si.on_wait = [si.on_wait[-1]]
                    n_fixed += 1
                i += 1
    return n_fixed


def _get_program(masked=True, legalized=True, debug=False):
    key = (masked, legalized, debug)
    if key not in _PROGRAMS:
        nc = _build_program(masked, debug=debug)
        if legalized:
            _legalize_sync_waits(nc)
        _PROGRAMS[key] = nc
    return _PROGRAMS[key]


def _make_in_maps(tokens_cont, mask, W1, b1, W2, b2):
    tokens = np.ascontiguousarray(tokens_cont, dtype=np.float32)
    mask_u8 = np.ascontiguousarray(mask).astype(np.uint8)
    isq2 = np.float32(1.0 / np.sqrt(2.0))
    w1 = np.ascontiguousarray(W1, dtype=np.float32) * isq2
    b1v = np.ascontiguousarray(b1, dtype=np.float32) * isq2
    w2 = np.ascontiguousarray(W2, dtype=np.float32)
    b2v = np.ascontiguousarray(b2, dtype=np.float32)
    in_maps = []
    for c in range(NCORES):
        sl = slice(NB * c, NB * (c + 1))
        in_maps.append(
            dict(
                tokens=np.ascontiguousarray(tokens[sl]),
                maskin=np.ascontiguousarray(mask_u8[sl]),
                w1=w1,
                b1=b1v,
                w2=w2,
                b2=b2v,
            )
        )
    return in_maps


def kernel(tokens_cont, mask, W1, b1, W2, b2):
    from concourse.bass_utils import run_bass_kernel_spmd

    nc = _get_program(masked=not np.all(mask))
    in_maps = _make_in_maps(tokens_cont, mask, W1, b1, W2, b2)
    res = run_bass_kernel_spmd(nc, in_maps, list(range(NCORES)))
    outs = [np.asarray(res.results[c]["out"]) for c in range(NCORES)]
    return np.concatenate(outs, axis=0).reshape(B, T, D)


# revision 14
# speedup vs baseline: 1.2236x; 1.2236x over previous
"""Trainium2 Bass kernel for nn_NodewiseMassBias.

Pipeline per core (2 batches of the 16, data-parallel over batch):
  for each query tile of 128 tokens:
    PE   : Lorentz-dot slab [128, 2048] (mask folded in as 5th contraction row)
    ACT  : slab PSUM -> SBUF
    DVE  : max8 (top-8 values per row) + max_index (their column indices)
    SWDGE: indirect gather of the 8 friends' masked 4-momenta from DRAM
    DVE  : prefix sums over sorted friends -> invariant masses for k=2,4,8
  then one Sqrt pass and a per-tile MLP (bias via extra rank-1 matmuls),
  GELU on ACT, output DMA per tile.
"""

import math

import numpy as np

NCORES = 8
B, T, D, H = 16, 2048, 512, 64
NB = B // NCORES  # batches per core
P = 128
NT = T // P
NK = 3
KV = (2, 4, 8)

_PROGRAMS = {}


def _build_program(masked, debug=False):
    import concourse.bass as bass
    import concourse.mybir as mybir
    from concourse.masks import make_identity
    from concourse.tile import TileContext

    f32 = mybir.dt.float32
    f32r = mybir.dt.float32r
    u8 = mybir.dt.uint8
    u32 = mybir.dt.uint32
    AF = mybir.ActivationFunctionType
    ALU = mybir.AluOpType
    AX = mybir.AxisListType

    LN_HALF = float(math.log(0.5))
    HALF_PI = float(math.pi / 2.0)
    PI_F = float(np.float32(math.pi))
    NEG_BIG = -1.0e30
    SQ2H = float(math.sqrt(2.0) / 2.0)

    nc = bass.Bass(num_swdge_queues=2)
    tokens = nc.dram_tensor("tokens", [NB, T, 4], f32, kind="ExternalInput")
    maskin = nc.dram_tensor("maskin", [NB, T], u8, kind="ExternalInput")
    w1 = nc.dram_tensor("w1", [NK, H], f32, kind="ExternalInput")
    b1 = nc.dram_tensor("b1", [H], f32, kind="ExternalInput")
    w2 = nc.dram_tensor("w2", [H, D], f32, kind="ExternalInput")
    b2 = nc.dram_tensor("b2", [D], f32, kind="ExternalInput")
    outd = nc.dram_tensor("out", [NB, T, D], f32, kind="ExternalOutput")
    p0d = [nc.dram_tensor(f"p0_{bb}", [T, 4], f32) for bb in range(NB)]
    if debug:
        dbg_maxv = nc.dram_tensor("dbg_maxv", [NB, NT, P, 8], f32, kind="ExternalOutput")
        dbg_idx = nc.dram_tensor("dbg_idx", [NB, NT, P, 8], u32, kind="ExternalOutput")
        dbg_pf = nc.dram_tensor("dbg_pf", [NB, NT, P, 32], f32, kind="ExternalOutput")
        dbg_mass = nc.dram_tensor("dbg_mass", [NB, T, NK], f32, kind="ExternalOutput")

    with TileContext(nc) as tc:
        with (
            tc.tile_pool(name="consts", bufs=1) as consts,
            tc.tile_pool(name="bwork", bufs=2) as bwork,
            tc.tile_pool(name="twork", bufs=2) as twork,
            tc.tile_pool(name="slabpool", bufs=2, space="PSUM") as slabpool,
            tc.tile_pool(name="mlppool", bufs=1, space="PSUM") as mlppool,
        ):
            ident = consts.tile([P, P], f32)
            make_identity(nc, ident)
            # ones row built on ACT (slab bias matmul then only waits ACT).
            ones1 = consts.tile([1, P], f32)
            nc.scalar.activation(ones1, ident[0:1, :], AF.Copy, scale=0.0, bias=1.0)

            _cbias_cache = {}

            def cbias(val, nparts=P):
                if val not in _cbias_cache:
                    t = consts.tile([P, 1], f32, tag=f"cbias_{len(_cbias_cache)}")
                    nc.vector.memset(t, float(val))
                    _cbias_cache[val] = t
                return _cbias_cache[val][:nparts, :]
            # Combined weight+bias operands (bias via a trailing ones row in
            # the activations), pre-rounded to f32r for single-pass matmuls.
            w1b1 = consts.tile([NK + 1, H], f32)
            nc.sync.dma_start(w1b1[0:NK, :], w1[:])
            nc.sync.dma_start(w1b1[NK : NK + 1, :], b1.rearrange("(o h) -> o h", o=1))
            w1b1r = consts.tile([NK + 1, H], f32r)
            nc.vector.tensor_copy(w1b1r, w1b1)
            w2b2 = consts.tile([H + 1, D], f32)
            nc.sync.dma_start(w2b2[0:H, :], w2[:])
            nc.sync.dma_start(w2b2[H : H + 1, :], b2.rearrange("(o d) -> o d", o=1))
            w2b2r = consts.tile([H + 1, D], f32r)
            nc.vector.tensor_copy(w2b2r, w2b2)
            # Warm-up matmuls: let PE observe each producer semaphore here,
            # one wait per instruction, so real matmuls carry <=1 wait
            # (PE LDWEIGHTS encodes at most one sync wait).
            warm = mlppool.tile([P, 128], f32, tag="tp")
            nc.tensor.transpose(warm[:P, :P], ident, ident)
            nc.tensor.matmul(
                warm[:H, :H], lhsT=w1b1r, rhs=w1b1r, start=True, stop=True
            )
            nc.tensor.matmul(
                warm[:P, :P], lhsT=w2b2r[:, 0:P], rhs=w2b2r[:, 0:P],
                start=True, stop=True,
            )

            for bi in range(NB):
                # ---- per-batch prep ----
                tok = bwork.tile([P, NT, 4], f32, tag="tok")
                nc.sync.dma_start(tok, tokens[bi].rearrange("(n p) c -> p n c", p=P))
                if masked:
                    mku = bwork.tile([P, NT], u8, tag="mku")
                    nc.sync.dma_start(mku, maskin[bi].rearrange("(n p) -> p n", p=P))
                    maskf = bwork.tile([P, NT], f32, tag="maskf")
                    nc.vector.tensor_copy(maskf, mku)
                    mrow_u = bwork.tile([1, T], u8, tag="mrowu")
                    nc.sync.dma_start(mrow_u, maskin[bi].rearrange("(o t) -> o t", o=1))
                    mrow_f = bwork.tile([1, T], f32, tag="mrowf")
                    nc.vector.tensor_copy(mrow_f, mrow_u)

                Pn = bwork.tile([P, NT, 4], f32, tag="Pn")
                P0 = bwork.tile([P, NT, 4], f32, tag="P0")
                etac = bwork.tile([P, NT], f32, tag="etac")
                expp = bwork.tile([P, NT], f32, tag="expp")
                expm = bwork.tile([P, NT], f32, tag="expm")
                cosp = bwork.tile([P, NT], f32, tag="cosp")
                sinp = bwork.tile([P, NT], f32, tag="sinp")
                shh = bwork.tile([P, NT], f32, tag="shh")
                phic = bwork.tile([P, NT], f32, tag="phic")
                aphi = bwork.tile([P, NT], f32, tag="aphi")

                nc.vector.tensor_scalar(
                    etac, tok[:, :, 2], -20.0, 20.0, op0=ALU.max, op1=ALU.min
                )
                nc.vector.tensor_scalar(
                    phic, tok[:, :, 3], -PI_F, PI_F, op0=ALU.max, op1=ALU.min
                )
                nc.scalar.activation(expp, etac, AF.Exp, bias=cbias(LN_HALF))
                nc.scalar.activation(expm, etac, AF.Exp, scale=-1.0, bias=cbias(LN_HALF))
                nc.scalar.activation(aphi, phic, AF.Abs)
                # cos(x) = sin(pi/2 - |x|), argument stays inside [-pi/2, pi/2]
                nc.scalar.activation(
                    cosp, aphi, AF.Sin, scale=-1.0, bias=cbias(HALF_PI)
                )
                nc.scalar.activation(sinp, phic, AF.Sin)
                nc.vector.tensor_copy(Pn[:, :, 0], tok[:, :, 0])
                nc.vector.tensor_mul(Pn[:, :, 1], tok[:, :, 1], cosp)
                nc.vector.tensor_mul(Pn[:, :, 2], tok[:, :, 1], sinp)
                nc.vector.tensor_sub(shh, expp, expm)
                nc.vector.tensor_mul(Pn[:, :, 3], tok[:, :, 1], shh)
                if masked:
                    for c in range(4):
                        nc.vector.tensor_mul(P0[:, :, c], Pn[:, :, c], maskf)
                    nc.sync.dma_start(p0d[bi].rearrange("(n p) c -> p n c", p=P), P0)
                else:
                    nc.sync.dma_start(p0d[bi].rearrange("(n p) c -> p n c", p=P), Pn)

                # Transposed layouts for the dot matmuls:
                # PTe = [E,px,py,pz], PTme = [E,-px,-py,-pz], both [4, T].
                Pnm = bwork.tile([P, NT, 4], f32, tag="Pnm")
                nc.vector.tensor_copy(Pnm[:, :, 0:1], Pn[:, :, 0:1])
                nc.vector.tensor_scalar_mul(Pnm[:, :, 1:4], Pn[:, :, 1:4], -1.0)
                PTe = bwork.tile([4, T], f32, tag="PTe")
                PTme = bwork.tile([4, T], f32, tag="PTme")
                for j in range(NT):
                    tpe = mlppool.tile([4, P], f32, tag="tp")
                    nc.tensor.transpose(tpe, Pn[:, j, :], ident)
                    nc.scalar.copy(PTe[:, j * P : (j + 1) * P], tpe)
                    tpm = mlppool.tile([4, P], f32, tag="tp")
                    nc.tensor.transpose(tpm, Pnm[:, j, :], ident)
                    nc.scalar.copy(PTme[:, j * P : (j + 1) * P], tpm)
                if masked:
                    # Fold the column mask into PTme's E row: E'_s = E_s - 1e30*(1-m_s)
                    # makes dot'[t,s] = dot[t,s] - E_t*1e30*(1-m_s) << any valid dot
                    # (E_t > 0), so masked s never enter the top-8.
                    mneg = bwork.tile([1, T], f32, tag="mneg")
                    nc.scalar.activation(
                        mneg, mrow_f, AF.Identity, scale=1.0e30, bias=cbias(NEG_BIG, 1)
                    )
                    nc.vector.tensor_add(PTme[0:1, :], PTme[0:1, :], mneg)

                # ---- loop 1: dot slab, top-8, friend sums, masses ----
                m2all = bwork.tile([P, NT, NK], f32, tag="m2all")
                massall = bwork.tile([P, NT, NK + 1], f32, tag="massall")
                nc.vector.memset(massall[:, :, NK : NK + 1], 1.0)
                for i in range(NT):
                    ssb = twork.tile([P, T], f32, tag="ssb")
                    for h in range(2):
                        slab = slabpool.tile([P, T // 2], f32, tag="slab")
                        for j in range(2):
                            jj = h * 2 + j
                            nc.tensor.matmul(
                                slab[:, j * 512 : (j + 1) * 512],
                                lhsT=PTe[:, i * P : (i + 1) * P],
                                rhs=PTme[:, jj * 512 : (jj + 1) * 512],
                                start=True,
                                stop=True,
                            )
                        nc.scalar.copy(
                            ssb[:, h * (T // 2) : (h + 1) * (T // 2)], slab
                        )
                    mxv = twork.tile([P, 8], f32, tag="mxv")
                    nc.vector.max(out=mxv, in_=ssb)
                    idx = twork.tile([P, 8], u32, tag="idx")
                    nc.vector.max_index(out=idx, in_max=mxv, in_values=ssb)
                    pf = twork.tile([P, 8, 4], f32, tag="pf")
                    # one indirect DMA per friend column: the [P,8,4]-out /
                    # [P,8]-offset form miscorresponds on hardware
                    for j in range(8):
                        ginst = nc.gpsimd.indirect_dma_start(
                            out=pf[:, j, :],
                            out_offset=None,
                            in_=p0d[bi][:],
                            in_offset=bass.IndirectOffsetOnAxis(
                                ap=idx[:, j : j + 1], axis=0
                            ),
                        )
                        if j % 2:
                            ginst.ins.queue = "qPoolDynamic1"

                    ps = twork.tile([P, NK, 4], f32, tag="ps")
                    for kk, k in enumerate(KV):
                        nc.vector.reduce_sum(
                            ps[:, kk, :],
                            pf[:, 0:k, :].rearrange("p j c -> p c j"),
                            axis=AX.X,
                        )
                    q = twork.tile([P, NK, 4], f32, tag="q")
                    nc.vector.tensor_mul(q, ps, ps)
                    s3 = twork.tile([P, NK], f32, tag="s3")
                    nc.vector.reduce_sum(s3, q[:, :, 1:4], axis=AX.X)
                    nc.vector.tensor_sub(m2all[:, i, :], q[:, :, 0], s3)
                    if debug:
                        nc.sync.dma_start(dbg_maxv[bi, i], mxv)
                        nc.sync.dma_start(dbg_idx[bi, i], idx)
                        nc.sync.dma_start(dbg_pf[bi, i], pf.rearrange("p j c -> p (j c)"))
                    if masked:
                        nc.vector.tensor_scalar(
                            m2all[:, i, :],
                            m2all[:, i, :],
                            0.0,
                            maskf[:, i : i + 1],
                            op0=ALU.max,
                            op1=ALU.mult,
                        )
                    else:
                        nc.vector.tensor_scalar_max(
                            m2all[:, i, :], m2all[:, i, :], 0.0
                        )
                nc.scalar.activation(
                    massall[:, :, 0:NK], m2all, AF.Sqrt, bias=cbias(1.0e-8)
                )
                if debug:
                    nc.sync.dma_start(
                        dbg_mass[bi].rearrange("(n p) k -> p n k", p=P), massall
                    )

                # ---- loop 2: MLP ----
                for i in range(NT):
                    mtp = mlppool.tile([NK + 1, P], f32, tag="tp")
                    nc.tensor.transpose(mtp, massall[:, i, :], ident)
                    mts = twork.tile([NK + 1, P], f32r, tag="mts")
                    nc.scalar.copy(mts, mtp)
                    h1p = mlppool.tile([P, H], f32, tag="h1p")
                    nc.tensor.matmul(h1p, lhsT=mts, rhs=w1b1r, start=True, stop=True)
                    # gelu(h) = h * (c + c*erf(h/sqrt2)); h1p holds h/sqrt2
                    # (W1,b1 are host-prescaled), c = sqrt(2)/2.
                    uclip = twork.tile([P, H], f32, tag="uclip")
                    nc.vector.tensor_scalar(
                        uclip, h1p, 4.0, -4.0, op0=ALU.min, op1=ALU.max
                    )
                    erf_t = twork.tile([P, H], f32, tag="erf_t")
                    nc.scalar.activation(erf_t, uclip, AF.Erf)
                    phi_t = twork.tile([P, H], f32, tag="phi_t")
                    nc.vector.tensor_scalar(
                        phi_t, erf_t, SQ2H, SQ2H, op0=ALU.mult, op1=ALU.add
                    )
                    hsb = twork.tile([P, H + 1], f32, tag="hsb")
                    nc.vector.memset(hsb[:, H : H + 1], 1.0)
                    nc.vector.tensor_mul(hsb[:, 0:H], h1p, phi_t)
                    htp = mlppool.tile([H + 1, P], f32, tag="htp")
                    nc.tensor.transpose(htp, hsb, ident)
                    hts = twork.tile([H + 1, P], f32r, tag="hts")
                    nc.scalar.copy(hts, htp)
                    op_ = mlppool.tile([P, D], f32, tag="opp")
                    nc.tensor.matmul(op_, lhsT=hts, rhs=w2b2r, start=True, stop=True)
                    osb = twork.tile([P, D], f32, tag="osb")
                    nc.scalar.copy(osb, op_)
                    nc.sync.dma_start(
                        outd[bi].rearrange("(n p) d -> p n d", p=P)[:, i, :], osb
                    )
    return nc


def _legalize_sync_waits(nc):
    """This toolchain's walrus accepts at most ONE sync wait per instruction.

    Hoist surplus waits onto same-engine NoOps inserted immediately before
    the offending instruction (engine streams execute in order, so the NoOp
    waits gate the instruction just the same).
    """
    import concourse.mybir as mybir

    n_fixed = 0
    for fn in nc.m.functions:
        for bb in fn.blocks:
            insts = bb.instructions
            i = 0
            while i < len(insts):
                inst = insts[i]
                si = getattr(inst, "sync_info", None)
                eng = getattr(inst, "engine", None)
                if (
                    si is not None
                    and si.on_wait
                    and len(si.on_wait) > 1
                    and eng is not None
                    and eng != mybir.EngineType.Unassigned
                ):
                    extra = list(si.on_wait[:-1])
                    for j, w in enumerate(extra):
                        nop = mybir.InstNoOp(
                            name=f"{inst.name}-wsplit{j}", ins=[], outs=[]
                        )
                        nop.engine = eng
                        nop.sync_info = mybir.SyncInfo(on_wait=[w], on_update=[])
                        insts.insert(i, nop)
                        i += 1
                    si.on_wait = [si.on_wait[-1]]
                    n_fixed += 1
                i += 1
    return n_fixed


def _get_program(masked=True, legalized=True, debug=False):
    key = (masked, legalized, debug)
    if key not in _PROGRAMS:
        nc = _build_program(masked, debug=debug)
        if legalized:
            _legalize_sync_waits(nc)
        _PROGRAMS[key] = nc
    return _PROGRAMS[key]


def _make_in_maps(tokens_cont, mask, W1, b1, W2, b2):
    tokens = np.ascontiguousarray(tokens_cont, dtype=np.float32)
    mask_u8 = np.ascontiguousarray(mask).astype(np.uint8)
    isq2 = np.float32(1.0 / np.sqrt(2.0))
    w1 = np.ascontiguousarray(W1, dtype=np.float32) * isq2
    b1v = np.ascontiguousarray(b1, dtype=np.float32) * isq2
    w2 = np.ascontiguousarray(W2, dtype=np.float32)
    b2v = np.ascontiguousarray(b2, dtype=np.float32)
    in_maps = []
    for c in range(NCORES):
        sl = slice(NB * c, NB * (c + 1))
        in_maps.append(
            dict(
                tokens=np.ascontiguousarray(tokens[sl]),
                maskin=np.ascontiguousarray(mask_u8[sl]),
                w1=w1,
                b1=b1v,
                w2=w2,
                b2=b2v,
            )
        )
    return in_maps


def kernel(tokens_cont, mask, W1, b1, W2, b2):
    from concourse.bass_utils import run_bass_kernel_spmd

    nc = _get_program(masked=not np.all(mask))
    in_maps = _make_in_maps(tokens_cont, mask, W1, b1, W2, b2)
    res = run_bass_kernel_spmd(nc, in_maps, list(range(NCORES)))
    outs = [np.asarray(res.results[c]["out"]) for c in range(NCORES)]
    return np.concatenate(outs, axis=0).reshape(B, T, D)
